# revision 27
# baseline (speedup 1.0000x reference)
"""Multi-head attention (B=4, N=2048, C=1024, H=16, D=64) on 8 trn2 cores.

Sharding: core c handles batch b = c//2 and head-group g = c%2 (8 heads,
512 channels). No collectives: the row-parallel output projection partials
are summed on host (2 cores per batch), with bp + bv@Wp folded in on host
(softmax rows sum to 1, so the v-bias contributes a constant per channel).

Device layout is transposed end-to-end (tokens on the free axis):
  xT [C, N] -> Q^T/K^T pair tiles [128, N] (rows 0:64 head 2p, 64:128 head
  2p+1) -> S^T tiles [keys, queries] via K=64 matmuls -> exp on ACT (no max
  subtraction; scores are O(1) by construction) -> PV with V_aug
  [vA | ones64 | vB] per pair: psum rows split into O_u and a 64-row
  replicated rowsum -> DVE reciprocal * mul -> O^T -> Y^T = Wp_g^T O^T.

Schedule: attention is ACT(exp)-bound; the PE clock gate (HAM) re-throttles
to 1.2 GHz whenever the PE idles, so the flat (pair, query-chunk, key-tile)
pipeline keeps the PE saturated: S(step i+1) is emitted before PV(step i),
and the next pair's Q/K projection matmuls (plus, on the last pair, the
output projection) are interleaved as PE filler. PSUM: S 2x[128,1024] (4
banks) + 2 O-accumulators (2) + filler (2) = 8.

Matmul operands bf16 by default (KERNEL_MM_DT=f32r for ~1e-4 accuracy at
lower speed); accumulation is fp32 in PSUM.
"""

import os
import sys

sys.path.insert(0, "/opt/trn_rl_repo")

import numpy as np

B, N, C, H = 4, 2048, 1024, 16
D = C // H
SCALE = D**-0.5
NCORES = 8
FC = 512  # channels per core
NP = 4  # head pairs per core
KT8 = C // 128  # contraction tiles
NCQ = N // 512  # n-chunks of 512
NMT = N // 128  # key tiles

MM_DT = os.environ.get("KERNEL_MM_DT", "bf16")

_nc = None


def _cap(ap_slice, block_step, nblocks, width):
    """2-free-dim AP: nblocks blocks of `width` cols, stride block_step."""
    import concourse.bass as bass

    lst = [list(p) for p in ap_slice.ap]
    assert len(lst) == 2 and lst[1][0] == 1, lst
    return bass.AP(
        ap_slice.tensor, ap_slice.offset, [lst[0], [block_step, nblocks], [1, width]]
    )


def _build():
    import concourse.bacc as bacc
    import concourse.mybir as mybir
    import concourse.tile as tile

    F32 = mybir.dt.float32
    MDT = mybir.dt.bfloat16 if MM_DT == "bf16" else mybir.dt.float32r
    AF = mybir.ActivationFunctionType

    nc = bacc.Bacc("TRN2", target_bir_lowering=False, debug=False, num_devices=NCORES)

    xT_d = nc.dram_tensor("xT", (NCQ, 128, KT8 * 512), MDT, kind="ExternalInput").ap()
    wq_d = nc.dram_tensor("wq", (128, KT8 * FC), MDT, kind="ExternalInput").ap()
    wk_d = nc.dram_tensor("wk", (128, KT8 * FC), MDT, kind="ExternalInput").ap()
    wv_d = nc.dram_tensor("wv", (128, KT8 * FC), MDT, kind="ExternalInput").ap()
    wp_d = nc.dram_tensor("wp", (128, NP * C), MDT, kind="ExternalInput").ap()
    bq_d = nc.dram_tensor("bq", (128, NP), F32, kind="ExternalInput").ap()
    bk_d = nc.dram_tensor("bk", (128, NP), F32, kind="ExternalInput").ap()
    on_d = nc.dram_tensor("ones", (128, 256), MDT, kind="ExternalInput").ap()
    yT_d = nc.dram_tensor("yT", (C // 128, NCQ, 128, 512), F32, kind="ExternalOutput").ap()

    with tile.TileContext(nc) as tc:
        with (
            tc.tile_pool(name="sb", bufs=1) as sb,
            tc.tile_pool(name="pe_", bufs=4) as pbe,
            tc.tile_pool(name="prc", bufs=4) as prc,
            tc.tile_pool(name="pyb", bufs=4) as pyb,
            tc.tile_pool(name="psqk", bufs=2, space="PSUM") as psqk,
            tc.tile_pool(name="psa", bufs=2, space="PSUM") as psa,
            tc.tile_pool(name="pso", bufs=2, space="PSUM") as pso,
        ):
            # ---- resident tiles + DMAs ----
            QT = [sb.tile([128, N], MDT, name=f"qt{p}") for p in range(NP)]
            KT = [sb.tile([128, N], MDT, name=f"kt{p}") for p in range(NP)]
            VA = [sb.tile([128, 192 * NP], MDT, name=f"va{t}") for t in range(NMT)]
            OT = [sb.tile([128, N], MDT, name=f"ot{p}") for p in range(NP)]
            bq_t = sb.tile([128, NP], F32, name="bq_t")
            bk_t = sb.tile([128, NP], F32, name="bk_t")
            on_t = sb.tile([128, 256], MDT, name="on_t")
            nc.sync.dma_start(out=bq_t[:], in_=bq_d)
            nc.sync.dma_start(out=bk_t[:], in_=bk_d)
            nc.sync.dma_start(out=on_t[:], in_=on_d)
            # DMA priority: wq + xT chunk 0 first (first QK group), then wk
            # (S needs full K^T), wv, remaining xT, wp last. Host delivers
            # merged partition-major layouts so each is ONE contiguous DMA.
            def _wall(nm, src_ap):
                t = sb.tile([128, KT8 * FC], MDT, name=nm)
                nc.sync.dma_start(out=t[:], in_=src_ap)
                return t

            def _xall(ncq):
                t = sb.tile([128, KT8 * 512], MDT, name=f"xt_{ncq}")
                nc.sync.dma_start(out=t[:], in_=xT_d[ncq, :, :])
                return t

            wq_all = _wall("wq_all", wq_d)
            xt_all = [_xall(0)]
            wk_all = _wall("wk_all", wk_d)
            wv_all = _wall("wv_all", wv_d)
            for ncq in range(1, NCQ):
                xt_all.append(_xall(ncq))
            wp_all = sb.tile([128, NP * C], MDT, name="wp_all")
            nc.sync.dma_start(out=wp_all[:], in_=wp_d)

            # ---- QKV emission helpers ----
            def emit_qk_group(p, proj, ncq):
                """One 8-matmul psum group (+ DVE bias evac) for pair p.
                Returns list of closures emitting one instruction each."""
                w_all, bias_t, dst = (
                    (wq_all, bq_t, QT) if proj == 0 else (wk_all, bk_t, KT)
                )
                cs = slice(512 * ncq, 512 * (ncq + 1))
                state = {}

                def mk_mm(k):
                    def go():
                        if "pq" not in state:
                            state["pq"] = psqk.tile(
                                [128, 512], F32, tag="qk", name=f"pq_{p}_{proj}_{ncq}"
                            )
                        nc.tensor.matmul(
                            state["pq"][:],
                            w_all[:, FC * k + 128 * p : FC * k + 128 * (p + 1)],
                            xt_all[ncq][:, 512 * k : 512 * (k + 1)],
                            start=(k == 0), stop=(k == KT8 - 1), skip_group_check=True,
                        )

                    return go

                def evac():
                    nc.vector.tensor_scalar_add(
                        dst[p][:, cs], state["pq"][:], bias_t[:, p : p + 1]
                    )

                return [mk_mm(k) for k in range(KT8)] + [evac]

            def emit_v(nt):
                ncq, t = divmod(nt, 4)
                pv = psqk.tile([128, 512], F32, tag="qk", name=f"pv_{nt}")
                for k in range(KT8):
                    nc.tensor.matmul(
                        pv[:],
                        xt_all[ncq][:, 512 * k + 128 * t : 512 * k + 128 * (t + 1)],
                        wv_all[:, FC * k : FC * (k + 1)],
                        start=(k == 0), stop=(k == KT8 - 1),
                    )
                va = VA[nt]
                nc.vector.tensor_copy(
                    _cap(va[:, 0:64], 192, NP, 64), _cap(pv[:, 0:64], 128, NP, 64)
                )
                nc.vector.tensor_copy(
                    _cap(va[:, 128:192], 192, NP, 64), _cap(pv[:, 64:128], 128, NP, 64)
                )
                nc.vector.tensor_copy(_cap(va[:, 64:128], 192, NP, 64), on_t[:])

            # ---- output projection chunk emitter ----
            def emit_proj_group(c, ncq):
                """Y^T chunk: 4 accumulating matmuls + DVE evac + DMA out."""
                cs = slice(512 * ncq, 512 * (ncq + 1))
                state = {}

                def mk_mm(f):
                    def go():
                        if "py" not in state:
                            state["py"] = psqk.tile(
                                [128, 512], F32, tag="qk", name=f"py_{c}_{ncq}"
                            )
                        nc.tensor.matmul(
                            state["py"][:],
                            wp_all[:, C * f + 128 * c : C * f + 128 * (c + 1)],
                            OT[f][:, cs],
                            start=(f == 0), stop=(f == NP - 1), skip_group_check=True,
                        )

                    return go

                def evac():
                    yb = pyb.tile([128, 512], F32, tag="yb", name=f"yb_{c}_{ncq}")
                    nc.vector.tensor_copy(yb[:], state["py"][:])
                    nc.sync.dma_start(out=yT_d[c, ncq, :, :], in_=yb[:])

                return [mk_mm(f) for f in range(NP)] + [evac]

            # ---- prologue: QK for pair 0 + V, interleaved per n-chunk so the
            # PE works on chunk ncq while chunk ncq+1's xT tiles stream in ----
            for ncq in range(NCQ):
                for proj in range(2):
                    for go in emit_qk_group(0, proj, ncq):
                        go()
                for t in range(4):
                    emit_v(4 * ncq + t)

            # ---- attention strips with QKV filler ----
            # strip = (pair, 512-query-chunk). Both heads of the pair share one
            # [128, 1024] S psum tile: head A (rows 0:64 of K^T/Q^T) -> cols
            # 0:512, head B (rows 64:128) -> cols 512:1024; the two K=64
            # matmuls row-pack onto disjoint PE row-groups and run
            # concurrently. One exp covers both. Software-pipelined: S(mt+1)
            # is emitted before PV(mt) so the PE computes it while ACT does
            # exp(mt); QKV filler matmuls take the remaining PE slack.
            # HAM warm-up guard: the PE clock gate needs ~3.4us of
            # uninterrupted matmul work to release 2.4GHz, and the attention
            # steady state is bistable (holds whichever state it enters in).
            # The prologue usually provides the burst, but unlucky DMA phasing
            # can fragment it; this scratch burst uses only long-resident
            # operands so nothing can interrupt it.
            warm = psqk.tile([128, 512], F32, tag="qk", name="warm")
            for w in range(20):
                nc.tensor.matmul(
                    warm[:], wv_all[:, 0:128], wv_all[:, 0:512],
                    start=True, stop=True, skip_group_check=True,
                )

            filler = []
            steps = [(p, qc, mt) for p in range(NP) for qc in range(NCQ) for mt in range(NMT)]

            def emit_S(p, qc, mt):
                qs = slice(512 * qc, 512 * (qc + 1))
                ms = slice(128 * mt, 128 * (mt + 1))
                sa = psa.tile([128, 1024], F32, tag="sa", name=f"sa_{p}_{qc}_{mt}")
                nc.tensor.matmul(
                    sa[:, 0:512], KT[p][0:64, ms], QT[p][0:64, qs],
                    start=True, stop=True,
                )
                nc.tensor.matmul(
                    sa[:, 512:1024], KT[p][64:128, ms], QT[p][64:128, qs],
                    start=True, stop=True,
                )
                return sa

            ots = None
            sa_next = emit_S(*steps[0])
            for i, (p, qc, mt) in enumerate(steps):
                if mt == 0:
                    # new pair: queue next pair's QKV (and, on the last pair,
                    # the finished column's projection) as PE filler
                    if qc == 0 and p + 1 < NP:
                        for proj in range(2):
                            for ncq in range(NCQ):
                                filler.extend(emit_qk_group(p + 1, proj, ncq))
                    ots = [
                        pso.tile([128, 512], F32, tag="o", name=f"o_{p}_{qc}_{j}")
                        for j in range(2)
                    ]
                sa_cur = sa_next
                ea = pbe.tile([128, 1024], MDT, tag="e", name=f"ea_{p}_{qc}_{mt}")
                nc.scalar.activation(ea[:], sa_cur[:], AF.Exp, scale=SCALE)
                if i + 1 < len(steps):
                    sa_next = emit_S(*steps[i + 1])
                for _ in range(4 if p == NP - 1 else 2):
                    if filler:
                        filler.pop(0)()
                first, last = mt == 0, mt == NMT - 1
                nc.tensor.matmul(
                    ots[0], VA[mt][:, 192 * p : 192 * p + 128], ea[:, 0:512],
                    start=first, stop=last, skip_group_check=True,
                )
                nc.tensor.matmul(
                    ots[1], VA[mt][:, 192 * p + 64 : 192 * p + 192], ea[:, 512:1024],
                    start=first, stop=last, skip_group_check=True,
                )
                if mt == NMT - 1:
                    qs = slice(512 * qc, 512 * (qc + 1))
                    for j in range(2):
                        o = ots[j]
                        # reciprocal_approx_fast mis-executes at base partition
                        # != 0: run it over the whole tile (unused rows produce
                        # garbage that is never read) and slice after.
                        rc = prc.tile([128, 512], F32, tag="rc", name=f"rc_{p}_{qc}_{j}")
                        nc.vector.reciprocal_approx_fast(rc[:], o[:])
                        osl, rcl = (
                            (o[0:64, :], rc[64:128, :]) if j == 0 else (o[64:128, :], rc[0:64, :])
                        )
                        nc.vector.tensor_mul(OT[p][64 * j : 64 * j + 64, qs], osl, rcl)
                    if p == NP - 1:
                        for c in range(C // 128):
                            filler.extend(emit_proj_group(c, qc))
            while filler:
                filler.pop(0)()

    nc.compile()
    return nc


def _get_nc():
    global _nc
    if _nc is None:
        try:
            import jax

            jax.config.update(
                "jax_compilation_cache_dir", os.path.expanduser("~/.cache/jax_bass")
            )
            jax.config.update("jax_persistent_cache_min_compile_time_secs", 0.0)
            jax.config.update("jax_persistent_cache_min_entry_size_bytes", 0)
        except Exception:
            pass
        _nc = _build()
    return _nc


def _wmerge(w, mdt):
    """(KT*128, F) -> [128, KT*F] partition-major merged layout."""
    kt = w.shape[0] // 128
    return np.ascontiguousarray(
        w.reshape(kt, 128, w.shape[1]).transpose(1, 0, 2).reshape(128, kt * w.shape[1]).astype(mdt)
    )


def make_in_maps(inputs):
    if MM_DT == "bf16":
        import ml_dtypes

        mdt = ml_dtypes.bfloat16
    else:
        mdt = np.float32
    x = np.asarray(inputs["x"], np.float32)
    Wq = np.asarray(inputs["Wq"], np.float32)
    Wk = np.asarray(inputs["Wk"], np.float32)
    Wv = np.asarray(inputs["Wv"], np.float32)
    Wp = np.asarray(inputs["Wp"], np.float32)
    bq = np.asarray(inputs["bq"], np.float32)
    bk = np.asarray(inputs["bk"], np.float32)
    ones = np.ones((128, 256), mdt)
    in_maps = []
    for core in range(NCORES):
        b, g = core // 2, core % 2
        sl = slice(FC * g, FC * (g + 1))
        in_maps.append(
            {
                "xT": np.ascontiguousarray(
                    x[b].T.reshape(KT8, 128, NCQ, 512)
                    .transpose(2, 1, 0, 3)
                    .reshape(NCQ, 128, KT8 * 512)
                    .astype(mdt)
                ),
                "wq": _wmerge(Wq[:, sl], mdt),
                "wk": _wmerge(Wk[:, sl], mdt),
                "wv": _wmerge(Wv[:, sl], mdt),
                "wp": _wmerge(Wp[sl, :], mdt),
                "bq": np.ascontiguousarray(bq[sl].reshape(NP, 128).T),
                "bk": np.ascontiguousarray(bk[sl].reshape(NP, 128).T),
                "ones": ones,
            }
        )
    return in_maps


def assemble(results, inputs):
    Wp = np.asarray(inputs["Wp"], np.float32)
    bv = np.asarray(inputs["bv"], np.float32)
    bp = np.asarray(inputs["bp"], np.float32)
    fb = (bp.astype(np.float64) + bv.astype(np.float64) @ Wp.astype(np.float64)).astype(
        np.float32
    )
    out = np.empty((B, N, C), np.float32)
    for b in range(B):
        yt = (results[2 * b]["yT"] + results[2 * b + 1]["yT"]).transpose(0, 2, 1, 3)
        out[b] = yt.reshape(C, N).T + fb
    return out


def run_on_device(inputs, trace=False, tmpdir=None):
    from concourse.bass_utils import run_bass_kernel_spmd

    nc = _get_nc()
    res = run_bass_kernel_spmd(
        nc, make_in_maps(inputs), list(range(NCORES)), trace=trace, tmpdir=tmpdir
    )
    return assemble(res.results, inputs), res


def kernel(**inputs):
    out, _ = run_on_device(inputs)
    return out
